# revision 11
# baseline (speedup 1.0000x reference)
"""Trainium2 Bass kernel for nn_MoDEChameleonMLP (MoDE Chameleon MLP).

Math (per token n):
  gate = x@Wg.T + delta_g(x); up = x@Wu.T + delta_u(x)
  inter = silu(gate)*up
  out  = inter@Wd.T + delta_d(inter)
where delta(v) = mask ? 2*(v@vA.T)@vB.T : 2*sum_e softmax(v@router.T)_e (v@A_e.T)@B_e.T

Implementation: token(B*S)-sharding across 8 cores (512 tokens/core, no
collectives), PE-issue-roofline oriented:
  aux1:   P[128,T] = Acat.T @ x accumulated over h-blocks with the A-matrices
          STATIONARY (output lands directly in [rank/logit, token] layout --
          no transposes). Routing softmax runs via two tiny select/broadcast
          matmuls + ACT/DVE ops, interleaved into the phase-1 stream so the
          PE never waits on it.
          y rows: 0:40 gate-delta, 64:104 up-delta (row-tile packing layout).
  phase1: per 128-row i-super: 32 Wg + 32 Wu matmuls (N=512) interleaved so
          consecutive matmuls alternate PSUM banks, plus ONE pair of K=64
          row-tiled B-delta matmuls that run CONCURRENTLY on disjoint PE row
          groups. silu*up -> inter resident in SBUF (bf16).
  aux2:   rank/logit projections of inter accumulate into P2 one matmul per
          super, interleaved into the phase-1 loop.
  phase2: down projection streaming Wd as the moving operand; the first
          h-super's PSUM comes from the just-freed phase-1 banks and the
          down-routing chain is interleaved into its matmul stream, so the
          phase boundary costs ~1us; B-delta via row-tiled pairs; outputs
          copied alternately on DVE/ACT engines.
Startup: xt is DMA'd in 8 chunks on the sync queue while the scalar-engine
HWDGE queue prefetches the A-matrices and the first two supers' weights;
the PE starts as soon as the first chunks land.
All matmuls bf16 with fp32 PSUM accumulation; weights are host-side
transposed/pre-tiled so every device DMA is wide contiguous lines.
"""
import os
import sys

for p in ("/root/.axon_site/_ro/trn_rl_repo", "/opt/trn_rl_repo"):
    if os.path.isdir(p) and p not in sys.path:
        sys.path.append(p)

import numpy as np
import ml_dtypes

import concourse.bass as bass  # noqa: E402
import concourse.tile as tile  # noqa: E402
from concourse import bacc, mybir  # noqa: E402
from concourse.bass_utils import run_bass_kernel_spmd  # noqa: E402

BF16 = ml_dtypes.bfloat16
BF = mybir.dt.bfloat16
F32 = mybir.dt.float32

NCORES = 8
T = 512          # tokens per core
TT = T // 128
E, R = 4, 8
SCALE = 2.0

_nc_cache = {}


def build_kernel(H, I):  # noqa: C901
    HB, IB = H // 128, I // 128
    NS, HS = IB, H // 512
    XC = 8            # xt dma chunks (HB/XC h-blocks each)
    WQ = 4            # weight dma chunks per (super, proj)
    HQ = HB // WQ     # h-blocks per weight chunk

    nc = bacc.Bacc(None, target_bir_lowering=False)
    xt_d = nc.declare_dram_parameter("xt", [128, HB, T], BF, isOutput=False)
    maskc_d = nc.declare_dram_parameter("maskc", [3, T], BF, isOutput=False)
    acall_d = nc.declare_dram_parameter("acall", [128, HB, 128], BF, isOutput=False)
    acd_d = nc.declare_dram_parameter("acd", [128, IB, 128], BF, isOutput=False)
    sels_d = nc.declare_dram_parameter("sels", [128, 67], BF, isOutput=False)
    selb_d = nc.declare_dram_parameter("selb", [128, 4, 128], BF, isOutput=False)
    wg_d = nc.declare_dram_parameter("wg", [NS, 128, HB, 128], BF, isOutput=False)
    wu_d = nc.declare_dram_parameter("wu", [NS, 128, HB, 128], BF, isOutput=False)
    bgu_d = nc.declare_dram_parameter("bgu", [NS, 128, 128], BF, isOutput=False)
    wd_d = nc.declare_dram_parameter("wd", [HS, 128, IB, 512], BF, isOutput=False)
    bd_d = nc.declare_dram_parameter("bd", [HS, 128, 512], BF, isOutput=False)
    out_d = nc.declare_dram_parameter("out", [T, H], F32, isOutput=True)

    AF = mybir.ActivationFunctionType
    OP = mybir.AluOpType

    ctx = tile.TileContext(nc)
    tc = ctx.__enter__()

    # SBUF pools spanning the whole kernel
    _sbuf_cms = [tc.tile_pool(name=n, bufs=b) for n, b in
                 (("const", 1), ("wstr", 12), ("wstr2", 10), ("bgustr", 4),
                  ("bstr2", 2), ("etmp", 2))]
    constp, wstr, wstr2, bgustr, bstr2, etmp = \
        [cm.__enter__() for cm in _sbuf_cms]

    # ---- HAM warm-up: ~3us of dummy matmuls on a memset tile keep the PE
    # busy during the startup DMA window so the clock gate opens (K=8/8)
    # before the first real matmul issues.
    warm_sb = constp.tile([128, 128], BF)
    nc.gpsimd.memset(warm_sb[:], 0.0)

    # ---- startup DMAs.  Super 0 consumes xt + acall + wg0/wu0 (7 MB) in
    # ~21us of matmul slots -- right at the per-core HBM roofline.  Both
    # HWDGE queues stream it demand-ordered in 4-h-block windows (sync:
    # xt + wu0, scalar: acall + wg0) so each window lands just before its
    # matmuls issue; super-1's weights follow on the scalar queue and the
    # big acd block rides the otherwise-idle SWDGE (gpsimd) path.
    acall_sb = constp.tile([128, HB, 128], BF)
    xt_sb = constp.tile([128, HB, T], BF)
    w01 = {}
    for proj in ("g", "u"):
        for q in range(WQ):
            w01[(proj, 0, q)] = wstr.tile([128, HQ, 128], BF, tag="wt",
                                          name=f"w{proj}0_{q}")
    # sync: xt in large (efficient) chunks, first one split for latency
    for lo, hi in ((0, 1), (1, 4), (4, 8), (8, 16), (16, 24), (24, 32)):
        nc.sync.dma_start(xt_sb[:, lo:hi, :], xt_d[:, lo:hi, :])
    # scalar: acall + wg0 demand-ordered; gpsimd (SWDGE): wu0 in parallel
    nc.scalar.dma_start(acall_sb[:, 0:2, :], acall_d[:, 0:2, :])
    nc.scalar.dma_start(w01[("g", 0, 0)][:, 0:2, :], wg_d[0, :, 0:2, :])
    nc.gpsimd.dma_start(w01[("u", 0, 0)][:, 0:2, :], wu_d[0, :, 0:2, :])
    nc.scalar.dma_start(acall_sb[:, 2:8, :], acall_d[:, 2:8, :])
    nc.scalar.dma_start(w01[("g", 0, 0)][:, 2:8, :], wg_d[0, :, 2:8, :])
    nc.gpsimd.dma_start(w01[("u", 0, 0)][:, 2:8, :], wu_d[0, :, 2:8, :])
    for q in range(1, WQ):
        lo, hi = q * HQ, (q + 1) * HQ
        nc.scalar.dma_start(acall_sb[:, lo:hi, :], acall_d[:, lo:hi, :])
        nc.scalar.dma_start(w01[("g", 0, q)][:], wg_d[0, :, lo:hi, :])
        nc.gpsimd.dma_start(w01[("u", 0, q)][:], wu_d[0, :, lo:hi, :])

    # super-1 weights follow on q2; bgu consts + acd trail
    for q in range(WQ):
        for proj, w_dram in (("g", wg_d), ("u", wu_d)):
            wq = wstr.tile([128, HQ, 128], BF, tag="wt", name=f"w{proj}1_{q}")
            nc.scalar.dma_start(wq[:], w_dram[1, :, q * HQ:(q + 1) * HQ, :])
            w01[(proj, 1, q)] = wq
    for s in (0, 1):
        bgt = bgustr.tile([128, 128], BF, tag="bgu", name=f"bgu{s}")
        nc.scalar.dma_start(bgt[:], bgu_d[s])
        w01[("b", s)] = bgt
    acd_sb = constp.tile([128, IB, 128], BF)
    nc.scalar.dma_start(acd_sb[:], acd_d[:])

    # small consts ride the sync queue behind the xt/wu0 windows
    sels_sb = constp.tile([128, 67], BF)
    nc.sync.dma_start(sels_sb[:], sels_d[:])
    selb_sb = constp.tile([128, 4, 128], BF)
    nc.sync.dma_start(selb_sb[:], selb_d[:])
    m1_sb = constp.tile([2, T], BF)
    nc.sync.dma_start(m1_sb[:], maskc_d[1:3, :])

    # routing scratch (shared by aux1/aux2)
    esb = constp.tile([128, T], BF)       # exp(P[32:64])
    w8sb = constp.tile([128, T], BF)      # combine weights, rows 32:64
    rcs = constp.tile([2, T], BF)         # reciprocal of softmax sums
    rcs2 = constp.tile([3, T], BF)        # rows 0:2 rec*(1-m); row 2 = m
    nc.sync.dma_start(rcs2[2:3, :], maskc_d[0:1, :])
    ssb = constp.tile([128, T], BF)       # staged S scale matrix
    yguT = constp.tile([128, T], BF)      # y rows 0:40 gate, 64:104 up
    ydT2 = constp.tile([128, T], BF)      # y_down rows 0:40 and 64:104
    inter_sb = constp.tile([128, IB, T], BF)

    ostp = tc.tile_pool(name="ost", bufs=4)
    ost = ostp.__enter__()

    # PSUM pools.  Bank budget: mainp(4) + p2p(1) + rtp(1) + aux1p(2) = 8.
    # Stack (LIFO) order: aux1p pops first (end of aux1), then at the end of
    # phase-2's first super: aux2p, rtp, p2p pop and the ops pool pushes.
    mp = tc.tile_pool(name="mainp", bufs=2, space="PSUM")
    mainp = mp.__enter__()
    p2p = tc.tile_pool(name="p2p", bufs=1, space="PSUM")
    p2pool = p2p.__enter__()
    rtp = tc.tile_pool(name="rtp", bufs=1, space="PSUM")
    rtpool = rtp.__enter__()
    P2 = p2pool.tile([128, T], F32, tag="p2", name="P2")
    rt = rtpool.tile([128, T], F32, tag="rt", name="rt")
    a1p = tc.tile_pool(name="aux1p", bufs=1, space="PSUM")
    aux1p = a1p.__enter__()
    P = aux1p.tile([128, T], F32, tag="P", name="P")
    S1 = aux1p.tile([128, T], F32, tag="S1", name="S1")

    pgu = {}

    def new_ps(s, pool=None, tags=("pg", "pu")):
        pool = pool or mainp
        pgu[("g", s)] = pool.tile([128, T], F32, tag=tags[0], name=f"pg{s}")
        pgu[("u", s)] = pool.tile([128, T], F32, tag=tags[1], name=f"pu{s}")

    def w_mms_gu(s, wtiles, hook=None, aux=False):
        """64 interleaved main matmuls (bank-alternating) for super s.
        hook(q) emits routing steps between chunk groups."""
        pg, pu = pgu[("g", s)], pgu[("u", s)]
        for q in range(WQ):
            if hook is not None:
                hook(q)
            for hh in range(HQ):
                h = q * HQ + hh
                if aux:
                    nc.tensor.matmul(P, acall_sb[:, h, :], xt_sb[:, h, :],
                                     start=(h == 0), stop=(h == HB - 1))
                nc.tensor.matmul(pg, wtiles[("g", q)][:, hh, :],
                                 xt_sb[:, h, :], start=(h == 0), stop=False)
                nc.tensor.matmul(pu, wtiles[("u", q)][:, hh, :],
                                 xt_sb[:, h, :], start=(h == 0), stop=False)

    def b_pair(s):
        bgt = w01[("b", s)] if s <= 1 else pgu[("b", s)]
        nc.tensor.matmul(pgu[("g", s)], bgt[0:64, :], yguT[0:64, :],
                         start=False, stop=True)
        nc.tensor.matmul(pgu[("u", s)], bgt[64:128, :], yguT[64:128, :],
                         start=False, stop=True)

    def silu_mult(s):
        stt = etmp.tile([128, T], F32, tag="silu", name=f"si{s}")
        nc.scalar.activation(stt[:], pgu[("g", s)][:], AF.Silu)
        nc.vector.tensor_tensor(inter_sb[:, s, :], stt[:], pgu[("u", s)][:],
                                OP.mult)
        del pgu[("g", s)], pgu[("u", s)]

    def aux2_mm(i):
        nc.tensor.matmul(P2, acd_sb[:, i, :], inter_sb[:, i, :],
                         start=(i == 0), stop=(i == IB - 1))

    def routing_steps(P_ap, nsum, o2col, qcol, pwplane, y_out, nrows, Sps):
        """4-step softmax-combine chain; caller interleaves main matmuls
        between steps.  Step k emits at most one PE matmul."""
        def step0():
            nc.scalar.activation(esb[32:64, :], P_ap[32:64, :], AF.Exp)
            nc.tensor.matmul(rt[0:nsum, :],
                             sels_sb[32:64, o2col:o2col + nsum],
                             esb[32:64, :], start=True, stop=True)

        def step0b():
            # slow DVE special function ([nsum,512] = serial per lane);
            # scheduled with a full chunk-group of PE slack
            with nc.allow_low_precision(reason="softmax scales; bf16 ok"):
                nc.vector.reciprocal(rcs[0:nsum, :], rt[0:nsum, :])
            nc.vector.tensor_tensor(rcs2[0:nsum, :], rcs[0:nsum, :],
                                    m1_sb[0:nsum, :], OP.mult)

        def step1():
            # broadcast rec*(1-m) to rows 40:40+nsum*4; m -> row 48
            nc.tensor.matmul(rt[32:64, :], sels_sb[0:3, qcol:qcol + 32],
                             rcs2[0:3, :], start=True, stop=True)

        def step2():
            nc.vector.tensor_tensor(w8sb[32:64, :], esb[32:64, :],
                                    rt[32:64, :], OP.mult)
            nc.tensor.matmul(Sps[:, :], selb_sb[32:64, pwplane, :],
                             w8sb[32:64, :], start=True, stop=True)

        def step3():
            nc.vector.tensor_copy(ssb[0:nrows, :], Sps[0:nrows, :])
            nc.vector.tensor_tensor(y_out[0:nrows, :], P_ap[0:nrows, :],
                                    ssb[0:nrows, :], OP.mult)

        return [step0, step0b, step1, step2, step3]

    a2p_holder = [None, None]

    # ---- HAM warm-up burst (no data deps beyond the memset); runs while
    # the PE would otherwise idle waiting for the first xt/weight chunks.
    for k in range(12):
        nc.tensor.matmul(rt[:, 0:128], warm_sb[:], warm_sb[:],
                         start=(k == 0), stop=(k == 11))

    # ---- super 0: aux1 + Wg/Wu interleaved with xt chunk arrival
    new_ps(0)
    w_mms_gu(0, {(p, q): w01[(p, 0, q)] for p in "gu" for q in range(WQ)},
             aux=True)

    # ---- supers 1..NS-1
    for s in range(1, NS):
        if s >= 2:
            wt = {}
            for q in range(WQ):
                for proj, w_dram in (("g", wg_d), ("u", wu_d)):
                    wq = wstr.tile([128, HQ, 128], BF, tag="wt",
                                   name=f"w{proj}{s}_{q}")
                    nc.sync.dma_start(wq[:],
                                      w_dram[s, :, q * HQ:(q + 1) * HQ, :])
                    wt[(proj, q)] = wq
            bgt = bgustr.tile([128, 128], BF, tag="bgu", name=f"bgu{s}")
            nc.sync.dma_start(bgt[:], bgu_d[s])
            pgu[("b", s)] = bgt
        else:
            wt = {(p, q): w01[(p, s, q)] for p in "gu" for q in range(WQ)}
        new_ps(s)
        if s == 1:
            steps = routing_steps(P, 2, 0, 3, 1, yguT, 128, S1)
            # the whole softmax chain is spread across super-1's chunk
            # boundaries (exp+sum+recip at q0, then one PE step per chunk)
            # so yguT is ready before super-2 needs it.  The recip (slow
            # DVE special function) gets a full chunk-group of slack.
            def hook(q):
                if q == 0:
                    steps[0]()
                    steps[1]()
                elif q == 1:
                    steps[2]()
                elif q == 2:
                    steps[3]()
                elif q == 3:
                    steps[4]()
            w_mms_gu(s, wt, hook=hook)
        elif s == 2:
            # boundary: both b-delta/silu chains for supers 0 and 1 land
            # here, spread across the chunk hooks.
            def hook(q):
                if q == 1:
                    b_pair(0)
                elif q == 2:
                    silu_mult(0)
                    b_pair(1)
                elif q == 3:
                    silu_mult(1)
            w_mms_gu(s, wt, hook=hook)
        else:
            # steady state: b-delta + silu/mult for super s-1 ride super s,
            # so the psum release chain (PE b-pair -> ACT silu -> DVE mult)
            # completes a full super before the bank is reused at the
            # super-s+1 start; the aux2 matmul rides the q0 boundary.
            def hook(q, _s=s):
                if q == 0 and _s >= 3:
                    aux2_mm(_s - 3)
                elif q == 2:
                    b_pair(_s - 1)
                elif q == 3:
                    silu_mult(_s - 1)
            w_mms_gu(s, wt, hook=hook)
        if s == 2:
            # free aux1 psum banks; open the bank for the down-routing S
            a1p.__exit__(None, None, None)
            a2p_holder[0] = tc.tile_pool(name="aux2p", bufs=1, space="PSUM")
            a2p_holder[1] = a2p_holder[0].__enter__()

    b_pair(NS - 1)
    silu_mult(NS - 1)
    for i in range(IB - 3, IB):
        aux2_mm(i)

    # ---- phase 2.  hs=0 reuses the (instantly-free) mainp banks and the
    # down-routing chain interleaves into its matmul stream; the ops pool
    # (aliasing the routing banks) only serves hs>=1, ~19us later.
    aux2p = a2p_holder[1]
    Sd = aux2p.tile([128, T], F32, tag="Sd", name="Sd")
    dsteps = routing_steps(P2, 1, 2, 35, 2, ydT2, 128, Sd)

    opsp = None
    ops = None
    for hs in range(HS):
        bdt = bstr2.tile([128, 512], BF, tag="bd2", name=f"bd{hs}")
        nc.sync.dma_start(bdt[:], bd_d[hs])
        if hs % 2 == 0:
            pso = [mainp.tile([128, 512], F32, tag=("pg", "pu")[t % 2],
                              name=f"po{hs}_{t}") for t in range(TT)]
        else:
            pso = [ops.tile([128, 512], F32, tag=f"o{t}",
                            name=f"po{hs}_{t}") for t in range(TT)]
        for i in range(IB):
            if hs == 0 and i in (4, 10, 14, 18):
                if i == 4:
                    dsteps[0]()
                    dsteps[1]()
                elif i == 10:
                    dsteps[2]()
                elif i == 14:
                    dsteps[3]()
                else:
                    dsteps[4]()
            wdt = wstr2.tile([128, 512], BF, tag="wd2", name=f"wd{hs}_{i}")
            # split the dense wd stream's descriptor-issue load across both
            # HWDGE queues (sync + scalar) so neither queue's issue rate
            # falls behind the PE's consumption mid-super.
            (nc.sync if i % 2 == 0 else nc.scalar).dma_start(
                wdt[:], wd_d[hs, :, i, :])
            for t in range(TT):
                nc.tensor.matmul(pso[t],
                                 inter_sb[:, i, t * 128:(t + 1) * 128],
                                 wdt[:], start=(i == 0), stop=False)
        for t in range(TT):
            lo = 64 * (t % 2)
            nc.tensor.matmul(pso[t],
                             ydT2[lo:lo + 64, t * 128:(t + 1) * 128],
                             bdt[lo:lo + 64, :], start=False, stop=True)
        for t in range(TT):
            osb = ost.tile([128, 512], F32, tag="os", name=f"os{hs}_{t}")
            if t % 2 == 0:
                nc.vector.tensor_copy(osb[:], pso[t][:])
                eng = nc.sync
            else:
                nc.scalar.activation(osb[:], pso[t][:], AF.Copy)
                eng = nc.scalar
            eng.dma_start(
                out_d[t * 128:(t + 1) * 128, hs * 512:(hs + 1) * 512],
                osb[:])
        if hs == 0:
            # routing psums done -> free their banks for the ops pool
            a2p_holder[0].__exit__(None, None, None)
            rtp.__exit__(None, None, None)
            p2p.__exit__(None, None, None)
            opsp = tc.tile_pool(name="ops", bufs=1, space="PSUM")
            ops = opsp.__enter__()

    opsp.__exit__(None, None, None)
    mp.__exit__(None, None, None)
    ostp.__exit__(None, None, None)

    for cm in reversed(_sbuf_cms):
        cm.__exit__(None, None, None)
    ctx.__exit__(None, None, None)
    nc.finalize()
    return nc


def get_nc(H, I):
    key = (H, I)
    if key not in _nc_cache:
        _nc_cache[key] = build_kernel(H, I)
    return _nc_cache[key]


def _sel_consts():
    """Constant selection/broadcast matrices for the routing matmuls."""
    sels = np.zeros((128, 67), np.float32)
    for e in range(E):
        sels[40 + e, 0] = 1.0      # O2 col0: gate logit rows
        sels[44 + e, 1] = 1.0      # O2 col1: up logit rows
        sels[40 + e, 2] = 1.0      # Od: down logit rows
    sels[0, 3 + 8:3 + 12] = 1.0    # Q8 row0 -> partitions 40:44 (gate)
    sels[1, 3 + 12:3 + 16] = 1.0   # Q8 row1 -> partitions 44:48 (up)
    sels[2, 3 + 16] = 1.0          # m -> partition 48
    sels[0, 35 + 8:35 + 12] = 1.0  # Qd row0 -> partitions 40:44 (down)
    sels[2, 35 + 16] = 1.0         # m -> partition 48 (down)

    selb = np.zeros((128, 4, 128), np.float32)
    for e in range(E):
        selb[40 + e, 1, 8 + 8 * e:16 + 8 * e] = 1.0    # Pw aux1: gate experts
        selb[44 + e, 1, 72 + 8 * e:80 + 8 * e] = 1.0   # Pw aux1: up experts
        selb[40 + e, 2, 8 + 8 * e:16 + 8 * e] = 1.0    # Pw aux2: down experts
        selb[40 + e, 2, 72 + 8 * e:80 + 8 * e] = 1.0   # Pw aux2: dup rows
    selb[48, 1, 0:8] = 1.0         # m -> va rows (gate)
    selb[48, 1, 64:72] = 1.0       # m -> va rows (up)
    selb[48, 2, 0:8] = 1.0         # m -> va rows (down)
    selb[48, 2, 64:72] = 1.0       # m -> va dup rows (down)
    return sels.astype(BF16), selb.astype(BF16)


def _prep_weights(Wg, Wu, Wd, va_gate_A, va_gate_B, va_up_A, va_up_B,
                  va_down_A, va_down_B, router_gate, tm_gate_A, tm_gate_B,
                  router_up, tm_up_A, tm_up_B, router_down, tm_down_A, tm_down_B):
    I, H = Wg.shape
    HB, IB = H // 128, I // 128
    NS, HS = IB, H // 512

    def tile_w_ih(W):  # [I,H] -> [NS,128,HB,128]; w[s,p,h,c]=W[s*128+c, h*128+p]
        return np.ascontiguousarray(
            W.reshape(NS, 128, HB, 128).transpose(0, 3, 2, 1)).astype(BF16)

    def bcat_rows(vB, tB):  # [out,40] = [vB | tB_e] scaled
        return SCALE * np.concatenate([vB] + [tB[e] for e in range(E)], axis=1)

    # bgu: rows 0:40 gate Bcat.T, 64:104 up Bcat.T per 128-col super
    Bp = np.zeros((128, I), np.float32)
    Bp[0:40] = bcat_rows(va_gate_B, tm_gate_B).T
    Bp[64:104] = bcat_rows(va_up_B, tm_up_B).T
    bgu = np.ascontiguousarray(
        Bp.reshape(128, NS, 128).transpose(1, 0, 2)).astype(BF16)

    # bd: rows 0:40 AND 64:104 = down Bcat.T per 512-col super
    Bpd = np.zeros((128, H), np.float32)
    Bpd[0:40] = bcat_rows(va_down_B, tm_down_B).T
    Bpd[64:104] = Bpd[0:40]
    bd = np.ascontiguousarray(
        Bpd.reshape(128, HS, 512).transpose(1, 0, 2)).astype(BF16)

    # acall columns: 0:8 va_g, 8:40 tm_g, 40:44 r_g, 44:48 r_u,
    #                64:72 va_u, 72:104 tm_u
    A2 = np.zeros((128, H), np.float32)
    A2[0:8] = va_gate_A
    A2[8:40] = tm_gate_A.reshape(E * R, H)
    A2[40:44] = router_gate
    A2[44:48] = router_up
    A2[64:72] = va_up_A
    A2[72:104] = tm_up_A.reshape(E * R, H)
    acall = np.ascontiguousarray(
        A2.T.reshape(HB, 128, 128).transpose(1, 0, 2)).astype(BF16)

    # acd columns: 0:8 va_d, 8:40 tm_d, 40:44 r_d; 64:104 duplicate of
    # 0:40 so the routing multiply directly yields both row-copies of y_down
    Ad = np.zeros((128, I), np.float32)
    Ad[0:8] = va_down_A
    Ad[8:40] = tm_down_A.reshape(E * R, I)
    Ad[40:44] = router_down
    Ad[64:104] = Ad[0:40]
    acd = np.ascontiguousarray(
        Ad.T.reshape(IB, 128, 128).transpose(1, 0, 2)).astype(BF16)

    wd = np.ascontiguousarray(
        Wd.reshape(HS, 512, IB, 128).transpose(0, 3, 2, 1)).astype(BF16)

    sels, selb = _sel_consts()
    return {
        "acall": acall,
        "acd": acd,
        "sels": sels,
        "selb": selb,
        "wg": tile_w_ih(Wg),
        "wu": tile_w_ih(Wu),
        "bgu": bgu,
        "wd": wd,
        "bd": bd,
    }


def _prep_core_inputs(x, image_mask, weights, n_cores):
    Bb, S, H = x.shape
    HB = H // 128
    xf = np.asarray(x, np.float32).reshape(-1, H)
    m = np.asarray(image_mask).reshape(-1).astype(np.float32)
    in_maps = []
    for c in range(n_cores):
        sh = xf[c * T:(c + 1) * T]                      # [T,H]
        xt = np.ascontiguousarray(
            sh.T.reshape(HB, 128, T).transpose(1, 0, 2)).astype(BF16)
        mc = m[c * T:(c + 1) * T]                       # [T]
        maskc = np.ascontiguousarray(
            np.stack([mc, 1.0 - mc, 1.0 - mc])).astype(BF16)
        in_maps.append({"xt": xt, "maskc": maskc, **weights})
    return in_maps


def run(x, image_mask, weights_raw, trace=False):
    Bb, S, H = x.shape
    I = weights_raw["Wg"].shape[0]
    nc = get_nc(H, I)
    weights = _prep_weights(**weights_raw)
    in_maps = _prep_core_inputs(x, image_mask, weights, NCORES)
    res = run_bass_kernel_spmd(nc, in_maps, list(range(NCORES)), trace=trace)
    out = np.concatenate([r["out"] for r in res.results], axis=0)
    return out.reshape(Bb, S, H).astype(np.float32), res


def kernel(x, image_mask, Wg, Wu, Wd,
           va_gate_A, va_gate_B, va_up_A, va_up_B, va_down_A, va_down_B,
           router_gate, tm_gate_A, tm_gate_B,
           router_up, tm_up_A, tm_up_B,
           router_down, tm_down_A, tm_down_B):
    weights_raw = dict(
        Wg=np.asarray(Wg, np.float32), Wu=np.asarray(Wu, np.float32),
        Wd=np.asarray(Wd, np.float32),
        va_gate_A=np.asarray(va_gate_A), va_gate_B=np.asarray(va_gate_B),
        va_up_A=np.asarray(va_up_A), va_up_B=np.asarray(va_up_B),
        va_down_A=np.asarray(va_down_A), va_down_B=np.asarray(va_down_B),
        router_gate=np.asarray(router_gate), tm_gate_A=np.asarray(tm_gate_A),
        tm_gate_B=np.asarray(tm_gate_B),
        router_up=np.asarray(router_up), tm_up_A=np.asarray(tm_up_A),
        tm_up_B=np.asarray(tm_up_B),
        router_down=np.asarray(router_down), tm_down_A=np.asarray(tm_down_A),
        tm_down_B=np.asarray(tm_down_B),
    )
    out, _ = run(np.asarray(x), np.asarray(image_mask), weights_raw, trace=False)
    return out



# revision 13
# speedup vs baseline: 1.0004x; 1.0004x over previous
"""Trainium2 Bass kernel for nn_MoDEChameleonMLP (MoDE Chameleon MLP).

Math (per token n):
  gate = x@Wg.T + delta_g(x); up = x@Wu.T + delta_u(x)
  inter = silu(gate)*up
  out  = inter@Wd.T + delta_d(inter)
where delta(v) = mask ? 2*(v@vA.T)@vB.T : 2*sum_e softmax(v@router.T)_e (v@A_e.T)@B_e.T

Implementation: token(B*S)-sharding across 8 cores (512 tokens/core, no
collectives), PE-issue-roofline oriented:
  aux1:   P[128,T] = Acat.T @ x accumulated over h-blocks with the A-matrices
          STATIONARY (output lands directly in [rank/logit, token] layout --
          no transposes). Routing softmax runs via two tiny select/broadcast
          matmuls + ACT/DVE ops, interleaved into the phase-1 stream so the
          PE never waits on it.
          y rows: 0:40 gate-delta, 64:104 up-delta (row-tile packing layout).
  phase1: per 128-row i-super: 32 Wg + 32 Wu matmuls (N=512) interleaved so
          consecutive matmuls alternate PSUM banks, plus ONE pair of K=64
          row-tiled B-delta matmuls that run CONCURRENTLY on disjoint PE row
          groups. silu*up -> inter resident in SBUF (bf16).
  aux2:   rank/logit projections of inter accumulate into P2 one matmul per
          super, interleaved into the phase-1 loop.
  phase2: down projection streaming Wd as the moving operand; the first
          h-super's PSUM comes from the just-freed phase-1 banks and the
          down-routing chain is interleaved into its matmul stream, so the
          phase boundary costs ~1us; B-delta via row-tiled pairs; outputs
          copied alternately on DVE/ACT engines.
Startup: xt is DMA'd in 8 chunks on the sync queue while the scalar-engine
HWDGE queue prefetches the A-matrices and the first two supers' weights;
the PE starts as soon as the first chunks land.
All matmuls bf16 with fp32 PSUM accumulation; weights are host-side
transposed/pre-tiled so every device DMA is wide contiguous lines.
"""
import os
import sys

for p in ("/root/.axon_site/_ro/trn_rl_repo", "/opt/trn_rl_repo"):
    if os.path.isdir(p) and p not in sys.path:
        sys.path.append(p)

import numpy as np
import ml_dtypes

import concourse.bass as bass  # noqa: E402
import concourse.tile as tile  # noqa: E402
from concourse import bacc, mybir  # noqa: E402
from concourse.bass_utils import run_bass_kernel_spmd  # noqa: E402

BF16 = ml_dtypes.bfloat16
BF = mybir.dt.bfloat16
F32 = mybir.dt.float32

NCORES = 8
T = 512          # tokens per core
TT = T // 128
E, R = 4, 8
SCALE = 2.0

_nc_cache = {}


def build_kernel(H, I):  # noqa: C901
    HB, IB = H // 128, I // 128
    NS, HS = IB, H // 512
    XC = 8            # xt dma chunks (HB/XC h-blocks each)
    WQ = 4            # weight dma chunks per (super, proj)
    HQ = HB // WQ     # h-blocks per weight chunk

    nc = bacc.Bacc(None, target_bir_lowering=False)
    xt_d = nc.declare_dram_parameter("xt", [128, HB, T], BF, isOutput=False)
    maskc_d = nc.declare_dram_parameter("maskc", [3, T], BF, isOutput=False)
    acall_d = nc.declare_dram_parameter("acall", [128, HB, 128], BF, isOutput=False)
    acd_d = nc.declare_dram_parameter("acd", [128, IB, 128], BF, isOutput=False)
    sels_d = nc.declare_dram_parameter("sels", [128, 67], BF, isOutput=False)
    selb_d = nc.declare_dram_parameter("selb", [128, 4, 128], BF, isOutput=False)
    wg_d = nc.declare_dram_parameter("wg", [NS, 128, HB, 128], BF, isOutput=False)
    wu_d = nc.declare_dram_parameter("wu", [NS, 128, HB, 128], BF, isOutput=False)
    bgu_d = nc.declare_dram_parameter("bgu", [NS, 128, 128], BF, isOutput=False)
    wd_d = nc.declare_dram_parameter("wd", [HS, 128, IB, 512], BF, isOutput=False)
    bd_d = nc.declare_dram_parameter("bd", [HS, 128, 512], BF, isOutput=False)
    out_d = nc.declare_dram_parameter("out", [T, H], F32, isOutput=True)

    AF = mybir.ActivationFunctionType
    OP = mybir.AluOpType

    ctx = tile.TileContext(nc)
    tc = ctx.__enter__()

    # SBUF pools spanning the whole kernel
    _sbuf_cms = [tc.tile_pool(name=n, bufs=b) for n, b in
                 (("const", 1), ("wstr", 12), ("wstr2", 10), ("bgustr", 4),
                  ("bstr2", 2), ("etmp", 2))]
    constp, wstr, wstr2, bgustr, bstr2, etmp = \
        [cm.__enter__() for cm in _sbuf_cms]

    # ---- HAM warm-up: ~3us of dummy matmuls on a memset tile keep the PE
    # busy during the startup DMA window so the clock gate opens (K=8/8)
    # before the first real matmul issues.
    warm_sb = constp.tile([128, 128], BF)
    nc.gpsimd.memset(warm_sb[:], 0.0)

    # ---- startup DMAs.  Super 0 consumes xt + acall + wg0/wu0 (7 MB) in
    # ~21us of matmul slots -- right at the per-core HBM roofline.  Both
    # HWDGE queues stream it demand-ordered in 4-h-block windows (sync:
    # xt + wu0, scalar: acall + wg0) so each window lands just before its
    # matmuls issue; super-1's weights follow on the scalar queue and the
    # big acd block rides the otherwise-idle SWDGE (gpsimd) path.
    acall_sb = constp.tile([128, HB, 128], BF)
    xt_sb = constp.tile([128, HB, T], BF)
    w01 = {}
    for proj in ("g", "u"):
        for q in range(WQ):
            w01[(proj, 0, q)] = wstr.tile([128, HQ, 128], BF, tag="wt",
                                          name=f"w{proj}0_{q}")
    # sync: xt in large (efficient) chunks, first one split for latency
    for lo, hi in ((0, 1), (1, 4), (4, 8), (8, 16), (16, 24), (24, 32)):
        nc.sync.dma_start(xt_sb[:, lo:hi, :], xt_d[:, lo:hi, :])
    # scalar: acall + wg0 demand-ordered; gpsimd (SWDGE): wu0 in parallel
    nc.scalar.dma_start(acall_sb[:, 0:2, :], acall_d[:, 0:2, :])
    nc.scalar.dma_start(w01[("g", 0, 0)][:, 0:2, :], wg_d[0, :, 0:2, :])
    nc.gpsimd.dma_start(w01[("u", 0, 0)][:, 0:2, :], wu_d[0, :, 0:2, :])
    nc.scalar.dma_start(acall_sb[:, 2:8, :], acall_d[:, 2:8, :])
    nc.scalar.dma_start(w01[("g", 0, 0)][:, 2:8, :], wg_d[0, :, 2:8, :])
    nc.gpsimd.dma_start(w01[("u", 0, 0)][:, 2:8, :], wu_d[0, :, 2:8, :])
    for q in range(1, WQ):
        lo, hi = q * HQ, (q + 1) * HQ
        nc.scalar.dma_start(acall_sb[:, lo:hi, :], acall_d[:, lo:hi, :])
        nc.scalar.dma_start(w01[("g", 0, q)][:], wg_d[0, :, lo:hi, :])
        nc.gpsimd.dma_start(w01[("u", 0, q)][:], wu_d[0, :, lo:hi, :])

    # super-1 weights, bgu consts and acd trail on the SWDGE (gpsimd)
    # path: keeping these issue instructions OFF the scalar queue means
    # the ACT engine reaches the routing-softmax chain (exp/silu) the
    # moment its data dependencies clear instead of queueing behind DMA
    # descriptor pushes.
    for q in range(WQ):
        for proj, w_dram in (("g", wg_d), ("u", wu_d)):
            wq = wstr.tile([128, HQ, 128], BF, tag="wt", name=f"w{proj}1_{q}")
            nc.gpsimd.dma_start(wq[:], w_dram[1, :, q * HQ:(q + 1) * HQ, :])
            w01[(proj, 1, q)] = wq
    for s in (0, 1):
        bgt = bgustr.tile([128, 128], BF, tag="bgu", name=f"bgu{s}")
        nc.gpsimd.dma_start(bgt[:], bgu_d[s])
        w01[("b", s)] = bgt
    acd_sb = constp.tile([128, IB, 128], BF)
    nc.gpsimd.dma_start(acd_sb[:], acd_d[:])

    # small consts ride the sync queue behind the xt/wu0 windows
    sels_sb = constp.tile([128, 67], BF)
    nc.sync.dma_start(sels_sb[:], sels_d[:])
    selb_sb = constp.tile([128, 4, 128], BF)
    nc.sync.dma_start(selb_sb[:], selb_d[:])
    m1_sb = constp.tile([2, T], BF)
    nc.sync.dma_start(m1_sb[:], maskc_d[1:3, :])

    # routing scratch (shared by aux1/aux2)
    esb = constp.tile([128, T], BF)       # exp(P[32:64])
    w8sb = constp.tile([128, T], BF)      # combine weights, rows 32:64
    rcs = constp.tile([2, T], BF)         # reciprocal of softmax sums
    rcs2 = constp.tile([3, T], BF)        # rows 0:2 rec*(1-m); row 2 = m
    nc.sync.dma_start(rcs2[2:3, :], maskc_d[0:1, :])
    ssb = constp.tile([128, T], BF)       # staged S scale matrix
    yguT = constp.tile([128, T], BF)      # y rows 0:40 gate, 64:104 up
    ydT2 = constp.tile([128, T], BF)      # y_down rows 0:40 and 64:104
    inter_sb = constp.tile([128, IB, T], BF)

    ostp = tc.tile_pool(name="ost", bufs=4)
    ost = ostp.__enter__()

    # PSUM pools.  Bank budget: mainp(4) + p2p(1) + rtp(1) + aux1p(2) = 8.
    # Stack (LIFO) order: aux1p pops first (end of aux1), then at the end of
    # phase-2's first super: aux2p, rtp, p2p pop and the ops pool pushes.
    mp = tc.tile_pool(name="mainp", bufs=2, space="PSUM")
    mainp = mp.__enter__()
    p2p = tc.tile_pool(name="p2p", bufs=1, space="PSUM")
    p2pool = p2p.__enter__()
    rtp = tc.tile_pool(name="rtp", bufs=1, space="PSUM")
    rtpool = rtp.__enter__()
    P2 = p2pool.tile([128, T], F32, tag="p2", name="P2")
    rt = rtpool.tile([128, T], F32, tag="rt", name="rt")
    a1p = tc.tile_pool(name="aux1p", bufs=1, space="PSUM")
    aux1p = a1p.__enter__()
    P = aux1p.tile([128, T], F32, tag="P", name="P")
    S1 = aux1p.tile([128, T], F32, tag="S1", name="S1")

    pgu = {}

    def new_ps(s, pool=None, tags=("pg", "pu")):
        pool = pool or mainp
        pgu[("g", s)] = pool.tile([128, T], F32, tag=tags[0], name=f"pg{s}")
        pgu[("u", s)] = pool.tile([128, T], F32, tag=tags[1], name=f"pu{s}")

    def w_mms_gu(s, wtiles, hook=None, aux=False):
        """64 interleaved main matmuls (bank-alternating) for super s.
        hook(q) emits routing steps between chunk groups.  The aux1 P
        matmuls ride at double rate (2 per h for the first half) so P --
        and with it the whole routing-softmax chain -- completes as soon
        as xt has landed rather than at super-0's end."""
        pg, pu = pgu[("g", s)], pgu[("u", s)]
        for q in range(WQ):
            if hook is not None:
                hook(q)
            for hh in range(HQ):
                h = q * HQ + hh
                if aux and h < HB // 2:
                    nc.tensor.matmul(P, acall_sb[:, 2 * h, :],
                                     xt_sb[:, 2 * h, :],
                                     start=(h == 0), stop=False)
                    nc.tensor.matmul(P, acall_sb[:, 2 * h + 1, :],
                                     xt_sb[:, 2 * h + 1, :],
                                     start=False, stop=(2 * h + 1 == HB - 1))
                nc.tensor.matmul(pg, wtiles[("g", q)][:, hh, :],
                                 xt_sb[:, h, :], start=(h == 0), stop=False)
                nc.tensor.matmul(pu, wtiles[("u", q)][:, hh, :],
                                 xt_sb[:, h, :], start=(h == 0), stop=False)

    def b_pair(s):
        bgt = w01[("b", s)] if s <= 1 else pgu[("b", s)]
        nc.tensor.matmul(pgu[("g", s)], bgt[0:64, :], yguT[0:64, :],
                         start=False, stop=True)
        nc.tensor.matmul(pgu[("u", s)], bgt[64:128, :], yguT[64:128, :],
                         start=False, stop=True)

    def silu_mult(s):
        stt = etmp.tile([128, T], F32, tag="silu", name=f"si{s}")
        nc.scalar.activation(stt[:], pgu[("g", s)][:], AF.Silu)
        nc.vector.tensor_tensor(inter_sb[:, s, :], stt[:], pgu[("u", s)][:],
                                OP.mult)
        del pgu[("g", s)], pgu[("u", s)]

    def aux2_mm(i):
        nc.tensor.matmul(P2, acd_sb[:, i, :], inter_sb[:, i, :],
                         start=(i == 0), stop=(i == IB - 1))

    def routing_steps(P_ap, nsum, o2col, qcol, pwplane, y_out, nrows, Sps):
        """4-step softmax-combine chain; caller interleaves main matmuls
        between steps.  Step k emits at most one PE matmul."""
        def step0():
            nc.scalar.activation(esb[32:64, :], P_ap[32:64, :], AF.Exp)
            nc.tensor.matmul(rt[0:nsum, :],
                             sels_sb[32:64, o2col:o2col + nsum],
                             esb[32:64, :], start=True, stop=True)

        def step0b():
            # slow DVE special function ([nsum,512] = serial per lane);
            # scheduled with a full chunk-group of PE slack
            with nc.allow_low_precision(reason="softmax scales; bf16 ok"):
                nc.vector.reciprocal(rcs[0:nsum, :], rt[0:nsum, :])
            nc.vector.tensor_tensor(rcs2[0:nsum, :], rcs[0:nsum, :],
                                    m1_sb[0:nsum, :], OP.mult)

        def step1():
            # broadcast rec*(1-m) to rows 40:40+nsum*4; m -> row 48
            nc.tensor.matmul(rt[32:64, :], sels_sb[0:3, qcol:qcol + 32],
                             rcs2[0:3, :], start=True, stop=True)

        def step2():
            nc.vector.tensor_tensor(w8sb[32:64, :], esb[32:64, :],
                                    rt[32:64, :], OP.mult)
            nc.tensor.matmul(Sps[:, :], selb_sb[32:64, pwplane, :],
                             w8sb[32:64, :], start=True, stop=True)

        def step3():
            nc.vector.tensor_copy(ssb[0:nrows, :], Sps[0:nrows, :])
            nc.vector.tensor_tensor(y_out[0:nrows, :], P_ap[0:nrows, :],
                                    ssb[0:nrows, :], OP.mult)

        return [step0, step0b, step1, step2, step3]

    a2p_holder = [None, None]

    # ---- HAM warm-up burst (no data deps beyond the memset); runs while
    # the PE would otherwise idle waiting for the first xt/weight chunks.
    for k in range(12):
        nc.tensor.matmul(rt[:, 0:128], warm_sb[:], warm_sb[:],
                         start=(k == 0), stop=(k == 11))

    # ---- super 0: aux1 + Wg/Wu interleaved with xt chunk arrival
    new_ps(0)
    w_mms_gu(0, {(p, q): w01[(p, 0, q)] for p in "gu" for q in range(WQ)},
             aux=True)

    # ---- supers 1..NS-1
    for s in range(1, NS):
        if s >= 2:
            wt = {}
            for q in range(WQ):
                for proj, w_dram in (("g", wg_d), ("u", wu_d)):
                    wq = wstr.tile([128, HQ, 128], BF, tag="wt",
                                   name=f"w{proj}{s}_{q}")
                    nc.sync.dma_start(wq[:],
                                      w_dram[s, :, q * HQ:(q + 1) * HQ, :])
                    wt[(proj, q)] = wq
            bgt = bgustr.tile([128, 128], BF, tag="bgu", name=f"bgu{s}")
            nc.sync.dma_start(bgt[:], bgu_d[s])
            pgu[("b", s)] = bgt
        else:
            wt = {(p, q): w01[(p, s, q)] for p in "gu" for q in range(WQ)}
        new_ps(s)
        if s == 1:
            steps = routing_steps(P, 2, 0, 3, 1, yguT, 128, S1)
            # the whole softmax chain is spread across super-1's chunk
            # boundaries (exp+sum+recip at q0, then one PE step per chunk)
            # so yguT is ready before super-2 needs it.  The recip (slow
            # DVE special function) gets a full chunk-group of slack.
            def hook(q):
                if q == 0:
                    steps[0]()
                    steps[1]()
                elif q == 1:
                    steps[2]()
                elif q == 2:
                    steps[3]()
                elif q == 3:
                    steps[4]()
            w_mms_gu(s, wt, hook=hook)
        elif s == 2:
            # boundary: both b-delta/silu chains for supers 0 and 1 land
            # here, spread across the chunk hooks.
            def hook(q):
                if q == 1:
                    b_pair(0)
                elif q == 2:
                    silu_mult(0)
                    b_pair(1)
                elif q == 3:
                    silu_mult(1)
            w_mms_gu(s, wt, hook=hook)
        else:
            # steady state: b-delta + silu/mult for super s-1 ride super s,
            # so the psum release chain (PE b-pair -> ACT silu -> DVE mult)
            # completes a full super before the bank is reused at the
            # super-s+1 start; the aux2 matmul rides the q0 boundary.
            def hook(q, _s=s):
                if q == 0 and _s >= 3:
                    aux2_mm(_s - 3)
                elif q == 2:
                    b_pair(_s - 1)
                elif q == 3:
                    silu_mult(_s - 1)
            w_mms_gu(s, wt, hook=hook)
        if s == 2:
            # free aux1 psum banks; open the bank for the down-routing S
            a1p.__exit__(None, None, None)
            a2p_holder[0] = tc.tile_pool(name="aux2p", bufs=1, space="PSUM")
            a2p_holder[1] = a2p_holder[0].__enter__()

    b_pair(NS - 1)
    silu_mult(NS - 1)
    for i in range(IB - 3, IB):
        aux2_mm(i)

    # ---- phase 2.  hs=0 reuses the (instantly-free) mainp banks and the
    # down-routing chain interleaves into its matmul stream; the ops pool
    # (aliasing the routing banks) only serves hs>=1, ~19us later.
    aux2p = a2p_holder[1]
    Sd = aux2p.tile([128, T], F32, tag="Sd", name="Sd")
    dsteps = routing_steps(P2, 1, 2, 35, 2, ydT2, 128, Sd)

    opsp = None
    ops = None
    for hs in range(HS):
        bdt = bstr2.tile([128, 512], BF, tag="bd2", name=f"bd{hs}")
        nc.sync.dma_start(bdt[:], bd_d[hs])
        if hs % 2 == 0:
            pso = [mainp.tile([128, 512], F32, tag=("pg", "pu")[t % 2],
                              name=f"po{hs}_{t}") for t in range(TT)]
        else:
            pso = [ops.tile([128, 512], F32, tag=f"o{t}",
                            name=f"po{hs}_{t}") for t in range(TT)]
        for i in range(IB):
            if hs == 0 and i in (4, 10, 14, 18):
                if i == 4:
                    dsteps[0]()
                    dsteps[1]()
                elif i == 10:
                    dsteps[2]()
                elif i == 14:
                    dsteps[3]()
                else:
                    dsteps[4]()
            wdt = wstr2.tile([128, 512], BF, tag="wd2", name=f"wd{hs}_{i}")
            # split the dense wd stream's descriptor-issue load across both
            # HWDGE queues (sync + scalar) so neither queue's issue rate
            # falls behind the PE's consumption mid-super.
            (nc.sync if i % 2 == 0 else nc.scalar).dma_start(
                wdt[:], wd_d[hs, :, i, :])
            for t in range(TT):
                nc.tensor.matmul(pso[t],
                                 inter_sb[:, i, t * 128:(t + 1) * 128],
                                 wdt[:], start=(i == 0), stop=False)
        for t in range(TT):
            lo = 64 * (t % 2)
            nc.tensor.matmul(pso[t],
                             ydT2[lo:lo + 64, t * 128:(t + 1) * 128],
                             bdt[lo:lo + 64, :], start=False, stop=True)
        for t in range(TT):
            osb = ost.tile([128, 512], F32, tag="os", name=f"os{hs}_{t}")
            if t % 2 == 0:
                nc.vector.tensor_copy(osb[:], pso[t][:])
                eng = nc.sync
            else:
                nc.scalar.activation(osb[:], pso[t][:], AF.Copy)
                eng = nc.scalar
            eng.dma_start(
                out_d[t * 128:(t + 1) * 128, hs * 512:(hs + 1) * 512],
                osb[:])
        if hs == 0:
            # routing psums done -> free their banks for the ops pool
            a2p_holder[0].__exit__(None, None, None)
            rtp.__exit__(None, None, None)
            p2p.__exit__(None, None, None)
            opsp = tc.tile_pool(name="ops", bufs=1, space="PSUM")
            ops = opsp.__enter__()

    opsp.__exit__(None, None, None)
    mp.__exit__(None, None, None)
    ostp.__exit__(None, None, None)

    for cm in reversed(_sbuf_cms):
        cm.__exit__(None, None, None)
    ctx.__exit__(None, None, None)
    nc.finalize()
    return nc


def get_nc(H, I):
    key = (H, I)
    if key not in _nc_cache:
        _nc_cache[key] = build_kernel(H, I)
    return _nc_cache[key]


def _sel_consts():
    """Constant selection/broadcast matrices for the routing matmuls."""
    sels = np.zeros((128, 67), np.float32)
    for e in range(E):
        sels[40 + e, 0] = 1.0      # O2 col0: gate logit rows
        sels[44 + e, 1] = 1.0      # O2 col1: up logit rows
        sels[40 + e, 2] = 1.0      # Od: down logit rows
    sels[0, 3 + 8:3 + 12] = 1.0    # Q8 row0 -> partitions 40:44 (gate)
    sels[1, 3 + 12:3 + 16] = 1.0   # Q8 row1 -> partitions 44:48 (up)
    sels[2, 3 + 16] = 1.0          # m -> partition 48
    sels[0, 35 + 8:35 + 12] = 1.0  # Qd row0 -> partitions 40:44 (down)
    sels[2, 35 + 16] = 1.0         # m -> partition 48 (down)

    selb = np.zeros((128, 4, 128), np.float32)
    for e in range(E):
        selb[40 + e, 1, 8 + 8 * e:16 + 8 * e] = 1.0    # Pw aux1: gate experts
        selb[44 + e, 1, 72 + 8 * e:80 + 8 * e] = 1.0   # Pw aux1: up experts
        selb[40 + e, 2, 8 + 8 * e:16 + 8 * e] = 1.0    # Pw aux2: down experts
        selb[40 + e, 2, 72 + 8 * e:80 + 8 * e] = 1.0   # Pw aux2: dup rows
    selb[48, 1, 0:8] = 1.0         # m -> va rows (gate)
    selb[48, 1, 64:72] = 1.0       # m -> va rows (up)
    selb[48, 2, 0:8] = 1.0         # m -> va rows (down)
    selb[48, 2, 64:72] = 1.0       # m -> va dup rows (down)
    return sels.astype(BF16), selb.astype(BF16)


def _prep_weights(Wg, Wu, Wd, va_gate_A, va_gate_B, va_up_A, va_up_B,
                  va_down_A, va_down_B, router_gate, tm_gate_A, tm_gate_B,
                  router_up, tm_up_A, tm_up_B, router_down, tm_down_A, tm_down_B):
    I, H = Wg.shape
    HB, IB = H // 128, I // 128
    NS, HS = IB, H // 512

    def tile_w_ih(W):  # [I,H] -> [NS,128,HB,128]; w[s,p,h,c]=W[s*128+c, h*128+p]
        return np.ascontiguousarray(
            W.reshape(NS, 128, HB, 128).transpose(0, 3, 2, 1)).astype(BF16)

    def bcat_rows(vB, tB):  # [out,40] = [vB | tB_e] scaled
        return SCALE * np.concatenate([vB] + [tB[e] for e in range(E)], axis=1)

    # bgu: rows 0:40 gate Bcat.T, 64:104 up Bcat.T per 128-col super
    Bp = np.zeros((128, I), np.float32)
    Bp[0:40] = bcat_rows(va_gate_B, tm_gate_B).T
    Bp[64:104] = bcat_rows(va_up_B, tm_up_B).T
    bgu = np.ascontiguousarray(
        Bp.reshape(128, NS, 128).transpose(1, 0, 2)).astype(BF16)

    # bd: rows 0:40 AND 64:104 = down Bcat.T per 512-col super
    Bpd = np.zeros((128, H), np.float32)
    Bpd[0:40] = bcat_rows(va_down_B, tm_down_B).T
    Bpd[64:104] = Bpd[0:40]
    bd = np.ascontiguousarray(
        Bpd.reshape(128, HS, 512).transpose(1, 0, 2)).astype(BF16)

    # acall columns: 0:8 va_g, 8:40 tm_g, 40:44 r_g, 44:48 r_u,
    #                64:72 va_u, 72:104 tm_u
    A2 = np.zeros((128, H), np.float32)
    A2[0:8] = va_gate_A
    A2[8:40] = tm_gate_A.reshape(E * R, H)
    A2[40:44] = router_gate
    A2[44:48] = router_up
    A2[64:72] = va_up_A
    A2[72:104] = tm_up_A.reshape(E * R, H)
    acall = np.ascontiguousarray(
        A2.T.reshape(HB, 128, 128).transpose(1, 0, 2)).astype(BF16)

    # acd columns: 0:8 va_d, 8:40 tm_d, 40:44 r_d; 64:104 duplicate of
    # 0:40 so the routing multiply directly yields both row-copies of y_down
    Ad = np.zeros((128, I), np.float32)
    Ad[0:8] = va_down_A
    Ad[8:40] = tm_down_A.reshape(E * R, I)
    Ad[40:44] = router_down
    Ad[64:104] = Ad[0:40]
    acd = np.ascontiguousarray(
        Ad.T.reshape(IB, 128, 128).transpose(1, 0, 2)).astype(BF16)

    wd = np.ascontiguousarray(
        Wd.reshape(HS, 512, IB, 128).transpose(0, 3, 2, 1)).astype(BF16)

    sels, selb = _sel_consts()
    return {
        "acall": acall,
        "acd": acd,
        "sels": sels,
        "selb": selb,
        "wg": tile_w_ih(Wg),
        "wu": tile_w_ih(Wu),
        "bgu": bgu,
        "wd": wd,
        "bd": bd,
    }


def _prep_core_inputs(x, image_mask, weights, n_cores):
    Bb, S, H = x.shape
    HB = H // 128
    xf = np.asarray(x, np.float32).reshape(-1, H)
    m = np.asarray(image_mask).reshape(-1).astype(np.float32)
    in_maps = []
    for c in range(n_cores):
        sh = xf[c * T:(c + 1) * T]                      # [T,H]
        xt = np.ascontiguousarray(
            sh.T.reshape(HB, 128, T).transpose(1, 0, 2)).astype(BF16)
        mc = m[c * T:(c + 1) * T]                       # [T]
        maskc = np.ascontiguousarray(
            np.stack([mc, 1.0 - mc, 1.0 - mc])).astype(BF16)
        in_maps.append({"xt": xt, "maskc": maskc, **weights})
    return in_maps


def run(x, image_mask, weights_raw, trace=False):
    Bb, S, H = x.shape
    I = weights_raw["Wg"].shape[0]
    nc = get_nc(H, I)
    weights = _prep_weights(**weights_raw)
    in_maps = _prep_core_inputs(x, image_mask, weights, NCORES)
    res = run_bass_kernel_spmd(nc, in_maps, list(range(NCORES)), trace=trace)
    out = np.concatenate([r["out"] for r in res.results], axis=0)
    return out.reshape(Bb, S, H).astype(np.float32), res


def kernel(x, image_mask, Wg, Wu, Wd,
           va_gate_A, va_gate_B, va_up_A, va_up_B, va_down_A, va_down_B,
           router_gate, tm_gate_A, tm_gate_B,
           router_up, tm_up_A, tm_up_B,
           router_down, tm_down_A, tm_down_B):
    weights_raw = dict(
        Wg=np.asarray(Wg, np.float32), Wu=np.asarray(Wu, np.float32),
        Wd=np.asarray(Wd, np.float32),
        va_gate_A=np.asarray(va_gate_A), va_gate_B=np.asarray(va_gate_B),
        va_up_A=np.asarray(va_up_A), va_up_B=np.asarray(va_up_B),
        va_down_A=np.asarray(va_down_A), va_down_B=np.asarray(va_down_B),
        router_gate=np.asarray(router_gate), tm_gate_A=np.asarray(tm_gate_A),
        tm_gate_B=np.asarray(tm_gate_B),
        router_up=np.asarray(router_up), tm_up_A=np.asarray(tm_up_A),
        tm_up_B=np.asarray(tm_up_B),
        router_down=np.asarray(router_down), tm_down_A=np.asarray(tm_down_A),
        tm_down_B=np.asarray(tm_down_B),
    )
    out, _ = run(np.asarray(x), np.asarray(image_mask), weights_raw, trace=False)
    return out

